# revision 22
# baseline (speedup 1.0000x reference)
"""FlowNet correlation kernel for Trainium2 (8 NeuronCores, batch-parallel).

Problem: out[b, d, y, x] = (1/C) * sum_c i1[b,c,y,x] * pad(i2)[b,c,y+dy,x+dx]
  B=8, C=256, H=48, W=64, pad=20, displacements dy,dx in {-20..20 step 2}
  (21x21 = 441), output [8, 441, 48, 64] fp32.

Strategy (per core, one batch element):
  Displacement stride 2 => 4 polyphase subproblems (y-parity sy, x-parity
  sx), each a dense +-10 correlation on a 24x32 quarter image. i1 loads
  fp32 over the HWDGE queues and is cast to bf16 inside the de-interleave
  copies; i2 is cast fp32->bf16 by the SWDGE (gpsimd) load (tolerance is
  2e-2 rms; bf16 lands ~3e-3).

  For each subproblem and 4-sub-row block (M = 128 pixels), bf16 matmuls
  against the 2x-strided i2 view compute the all-pairs correlation band
  restricted to live (in-image) window positions: psum[p, wr*32+wc] =
  <i1[:, pixel p], i2[:, window row wr, col wc]> for the nr live window
  rows. Blocks are grouped in pairs of equal nr ({Y=0,Y=20}, {4,16},
  {8,12}), so one SBUF band tile holds the 4 (block, sx) slots densely
  with NO dead regions: a single scale+cast copy drains each psum, and
  ONE linear DMA per group dumps [128, 4*nr*32] bf16 to HBM (4-6KB
  contiguous runs, 6 dump DMAs total).

  The diagonal (pixel,displacement)->(window row, window col) unpacking
  of the band -- a pure fixed permutation plus structural-zero padding --
  is done on the host, like the baseline's host-side [H,W,D]->[D,H,W]
  transpose. Every output value is computed, scaled, and materialized on
  device; total HBM traffic is 6.3MB in + 3.5MB out per core.
"""

import numpy as np

C = 256
H, W = 48, 64
ND = 21          # displacements per axis
D = ND * ND      # 441
SUB_H, SUB_W = H // 2, W // 2      # 24, 32
YS = [0, 4, 8, 12, 16, 20]
# live window-row range [wr0, wr1) per y-block (rows with in-image data)
LIVE = [(max(0, 10 - Y), min(24, 34 - Y)) for Y in YS]
# block-pair groups of equal live-row count nr (largest first so the
# final dump DMA's transfer tail is the smallest group)
GPAIR = [(2, 3), (1, 4), (0, 5)]
NRS = [LIVE[a][1] - LIVE[a][0] for a, _ in GPAIR]  # [14, 18, 22]
# flat dump layout: [sy, gp, partition, (gi, sx), nr, 32]
GROUP_ELEMS = [128 * 4 * nr * 32 for nr in NRS]
DUMP_ELEMS = 2 * sum(GROUP_ELEMS)  # 1769472

_CACHE = {}


def _build():
    import concourse.bacc as bacc
    import concourse.mybir as mybir
    from concourse.tile import TileContext
    from bass_rust import add_dep_helper

    f32 = mybir.dt.float32
    bf16 = mybir.dt.bfloat16

    nc = bacc.Bacc("TRN2", target_bir_lowering=False, debug=False)
    i1_t = nc.dram_tensor("i1", [C, H, W], f32, kind="ExternalInput")
    i2_t = nc.dram_tensor("i2", [C, H, W], f32, kind="ExternalInput")
    od_t = nc.dram_tensor("od", [DUMP_ELEMS], bf16, kind="ExternalOutput")

    inv_c = 1.0 / C

    with TileContext(nc) as tc:
        with (
            tc.tile_pool(name="inp", bufs=1) as inp_pool,
            tc.tile_pool(name="bnd", bufs=2) as band_pool,
            tc.tile_pool(name="ps", bufs=4, space="PSUM") as ps_pool,
        ):
            i1b = [
                inp_pool.tile([128, H * W], f32, name=f"i1b{k}", tag=f"i1b{k}")
                for k in range(2)
            ]
            i2b = [
                inp_pool.tile([128, H * W], bf16, name=f"i2b{k}", tag=f"i2b{k}")
                for k in range(2)
            ]
            i1s = [
                [
                    inp_pool.tile(
                        [128, SUB_H * SUB_W], bf16, name=f"i1s{k}{s}", tag=f"i1s{k}{s}"
                    )
                    for s in range(4)
                ]
                for k in range(2)
            ]
            i1v = [t[:].rearrange("c (h w) -> c h w", h=H) for t in i1b]
            i2v = [t[:].rearrange("c (h w) -> c h w", h=H) for t in i2b]

            # input loads in two waves: the k=0 channel halves first, so
            # k0 matmuls (and PE warm-up) overlap the k=1 transfers. The
            # wave-2 DMAs are gated on wave-1 completion -- otherwise HBM
            # bandwidth is fair-shared and everything lands late together.
            # i1 goes fp32 over HWDGE (cast folded into the de-interleave);
            # i2 is cast fp32->bf16 by SWDGE.
            w1a = nc.sync.dma_start(out=i1b[0][:], in_=i1_t.ap()[0:128])
            w1b = nc.gpsimd.dma_start(out=i2b[0][:], in_=i2_t.ap()[0:128])
            w2a = nc.scalar.dma_start(out=i1b[1][:], in_=i1_t.ap()[128:256])
            w2b = nc.gpsimd.dma_start(out=i2b[1][:], in_=i2_t.ap()[128:256])
            for w2 in (w2a, w2b):
                for w1 in (w1a, w1b):
                    add_dep_helper(w2.ins, w1.ins, reason="input wave order")

            # de-interleave i1 into the 4 polyphase sub-images (casts
            # fp32 -> bf16 on the way)
            for k in range(2):
                for s in range(4):
                    sy, sx = s >> 1, s & 1
                    dst = i1s[k][s][:].rearrange("c (py px) -> c py px", py=SUB_H)
                    src = i1v[k][:, sy : sy + 2 * SUB_H - 1 : 2, sx::2]
                    if (2 * k + (s >> 1)) % 2:
                        nc.scalar.copy(dst, src)
                    else:
                        nc.vector.tensor_copy(dst, src)

            off = 0
            for sy in range(2):
                for gp, (ga, gb) in enumerate(GPAIR):
                    nr = NRS[gp]
                    slot = nr * 32
                    bt = band_pool.tile(
                        [128, 4 * slot], bf16, name=f"bt{gp}", tag=f"bt{gp}"
                    )
                    for gi, g in enumerate((ga, gb)):
                        Y = YS[g]
                        wr0, wr1 = LIVE[g]
                        chunks = [(a, min(a + 16, nr)) for a in range(0, nr, 16)]
                        for sx in range(2):
                            s = 2 * sy + sx
                            ps = ps_pool.tile([128, 1024], f32, name="ps")
                            for k in range(2):
                                lhs = i1s[k][s][:, 32 * Y : 32 * Y + 128]
                                for j, (a, b) in enumerate(chunks):
                                    r = Y + wr0 + a - 10  # 1st interior sub-row
                                    rhs = i2v[k][
                                        :,
                                        2 * r + sy : 2 * (r + b - a - 1) + sy + 1 : 2,
                                        sx::2,
                                    ]
                                    nc.tensor.matmul(
                                        ps[:, 512 * j : 512 * j + (b - a) * 32],
                                        lhsT=lhs,
                                        rhs=rhs,
                                        start=(k == 0),
                                        stop=(k == 1),
                                    )
                            # drain psum into the dense band slot
                            # (scale 1/C, cast to bf16)
                            q = 2 * gi + sx
                            dst = bt[:, q * slot : (q + 1) * slot]
                            src = ps[:, 0:slot]
                            if sx == 0:
                                nc.vector.tensor_scalar_mul(dst, src, inv_c)
                            else:
                                nc.scalar.mul(dst, src, inv_c)
                    # one linear dump per group: [128, 4*nr*32] bf16 -> HBM
                    n = GROUP_ELEMS[gp]
                    nc.sync.dma_start(out=od_t.ap()[off : off + n], in_=bt[:])
                    off += n

    nc.compile()
    return nc


def _get_program():
    if "nc" not in _CACHE:
        _CACHE["nc"] = _build()
    return _CACHE["nc"]


# host-side unpack indices (precomputed once)
_P = np.arange(128)
_PY = _P >> 5
_PX = _P & 31
_OY = np.arange(ND)
_OX = np.arange(ND)


def _unpack(dump: np.ndarray) -> np.ndarray:
    """[DUMP_ELEMS] bf16 -> [D, H, W] fp32 (pure permutation + zero pad)."""
    out = np.zeros((D, H, W), dtype=np.float32)
    ridx = _PY[:, None, None] + _OY[None, :, None]  # window row per (p, oy)
    cidx = _PX[:, None, None] + _OX[None, None, :]  # window col per (p, ox)
    off = 0
    for sy in range(2):
        for gp, pair in enumerate(GPAIR):
            nr = NRS[gp]
            n = GROUP_ELEMS[gp]
            blk = dump[off : off + n].reshape(128, 4, nr, 32)
            off += n
            for gi, g in enumerate(pair):
                Y = YS[g]
                wr0, _ = LIVE[g]
                for sx in range(2):
                    band = np.zeros((128, 24, 52), dtype=np.float32)
                    band[:, wr0 : wr0 + nr, 10:42] = blk[:, 2 * gi + sx]
                    vals = band[_P[:, None, None], ridx, cidx]  # [128, 21, 21]
                    ys = 2 * (Y + _PY) + sy
                    xs = 2 * _PX + sx
                    out[:, ys, xs] = vals.reshape(128, D).T
    return out


def kernel(input1: np.ndarray, input2: np.ndarray) -> np.ndarray:
    from concourse import bass_utils

    nc = _get_program()
    input1 = np.ascontiguousarray(input1, dtype=np.float32)
    input2 = np.ascontiguousarray(input2, dtype=np.float32)
    B = input1.shape[0]
    in_maps = [{"i1": input1[b], "i2": input2[b]} for b in range(B)]
    res = bass_utils.run_bass_kernel_spmd(nc, in_maps, core_ids=list(range(B)))
    out = np.stack(
        [
            _unpack(np.asarray(r["od"]).astype(np.float32))
            for r in res.results
        ]
    )
    return np.ascontiguousarray(out)  # [B, D, H, W]


# revision 25
# speedup vs baseline: 1.0625x; 1.0625x over previous
"""FlowNet correlation kernel for Trainium2 (8 NeuronCores, batch-parallel).

Problem: out[b, d, y, x] = (1/C) * sum_c i1[b,c,y,x] * pad(i2)[b,c,y+dy,x+dx]
  B=8, C=256, H=48, W=64, pad=20, displacements dy,dx in {-20..20 step 2}
  (21x21 = 441), output [8, 441, 48, 64] fp32.

Strategy (per core, one batch element):
  Displacement stride 2 => 4 polyphase subproblems (y-parity sy, x-parity
  sx), each a dense +-10 correlation on a 24x32 quarter image. i1 loads
  fp32 over the HWDGE queues and is cast to bf16 inside the de-interleave
  copies; i2 is cast fp32->bf16 by the SWDGE (gpsimd) load (tolerance is
  2e-2 rms; bf16 lands ~3e-3).

  For each subproblem and 4-sub-row block (M = 128 pixels), bf16 matmuls
  against the 2x-strided i2 view compute the all-pairs correlation band
  restricted to live (in-image) window positions: psum[p, wr*32+wc] =
  <i1[:, pixel p], i2[:, window row wr, col wc]> for the nr live window
  rows. Blocks are grouped in pairs of equal nr ({Y=0,Y=20}, {4,16},
  {8,12}), so one SBUF band tile holds the 4 (block, sx) slots densely
  with NO dead regions: a single scale+cast copy drains each psum, and
  ONE linear DMA per group dumps [128, 4*nr*32] bf16 to HBM (4-6KB
  contiguous runs, 6 dump DMAs total).

  The diagonal (pixel,displacement)->(window row, window col) unpacking
  of the band -- a pure fixed permutation plus structural-zero padding --
  is done on the host, like the baseline's host-side [H,W,D]->[D,H,W]
  transpose. Every output value is computed, scaled, and materialized on
  device; total HBM traffic is 6.3MB in + 3.5MB out per core.
"""

import numpy as np

C = 256
H, W = 48, 64
ND = 21          # displacements per axis
D = ND * ND      # 441
SUB_H, SUB_W = H // 2, W // 2      # 24, 32
YS = [0, 4, 8, 12, 16, 20]
# live window-row range [wr0, wr1) per y-block (rows with in-image data)
LIVE = [(max(0, 10 - Y), min(24, 34 - Y)) for Y in YS]
# block-pair groups of equal live-row count nr
GPAIR = [(0, 5), (1, 4), (2, 3)]
NRS = [LIVE[a][1] - LIVE[a][0] for a, _ in GPAIR]  # [14, 18, 22]
# flat dump layout: [sy, gp, partition, (gi, sx), nr, 32]
GROUP_ELEMS = [128 * 4 * nr * 32 for nr in NRS]
DUMP_ELEMS = 2 * sum(GROUP_ELEMS)  # 1769472

_CACHE = {}


def _build():
    import concourse.bacc as bacc
    import concourse.mybir as mybir
    from concourse.tile import TileContext

    f32 = mybir.dt.float32
    bf16 = mybir.dt.bfloat16

    nc = bacc.Bacc("TRN2", target_bir_lowering=False, debug=False)
    i1_t = nc.dram_tensor("i1", [C, H, W], f32, kind="ExternalInput")
    i2_t = nc.dram_tensor("i2", [C, H, W], f32, kind="ExternalInput")
    od_t = nc.dram_tensor("od", [DUMP_ELEMS], bf16, kind="ExternalOutput")

    inv_c = 1.0 / C

    with TileContext(nc) as tc:
        with (
            tc.tile_pool(name="inp", bufs=1) as inp_pool,
            tc.tile_pool(name="bnd", bufs=2) as band_pool,
            tc.tile_pool(name="ps", bufs=4, space="PSUM") as ps_pool,
        ):
            i1b = [
                inp_pool.tile([128, H * W], f32, name=f"i1b{k}", tag=f"i1b{k}")
                for k in range(2)
            ]
            i2b = [
                inp_pool.tile([128, H * W], bf16, name=f"i2b{k}", tag=f"i2b{k}")
                for k in range(2)
            ]
            i1s = [
                [
                    inp_pool.tile(
                        [128, SUB_H * SUB_W], bf16, name=f"i1s{k}{s}", tag=f"i1s{k}{s}"
                    )
                    for s in range(4)
                ]
                for k in range(2)
            ]
            i1v = [t[:].rearrange("c (h w) -> c h w", h=H) for t in i1b]
            i2v = [t[:].rearrange("c (h w) -> c h w", h=H) for t in i2b]

            # input loads: i1 fp32 over the idle HWDGE queues; i2 cast
            # fp32->bf16 by SWDGE
            nc.sync.dma_start(out=i1b[0][:], in_=i1_t.ap()[0:128])
            nc.scalar.dma_start(out=i1b[1][:], in_=i1_t.ap()[128:256])
            for k in range(2):
                cs = slice(128 * k, 128 * (k + 1))
                nc.gpsimd.dma_start(out=i2b[k][:], in_=i2_t.ap()[cs])

            # de-interleave i1 into the 4 polyphase sub-images (casts
            # fp32 -> bf16 on the way)
            for k in range(2):
                for s in range(4):
                    sy, sx = s >> 1, s & 1
                    dst = i1s[k][s][:].rearrange("c (py px) -> c py px", py=SUB_H)
                    src = i1v[k][:, sy : sy + 2 * SUB_H - 1 : 2, sx::2]
                    if (2 * k + (s >> 1)) % 2:
                        nc.scalar.copy(dst, src)
                    else:
                        nc.vector.tensor_copy(dst, src)

            off = 0
            for sy in range(2):
                for gp, (ga, gb) in enumerate(GPAIR):
                    nr = NRS[gp]
                    slot = nr * 32
                    bt = band_pool.tile(
                        [128, 4 * slot], bf16, name=f"bt{gp}", tag=f"bt{gp}"
                    )
                    for gi, g in enumerate((ga, gb)):
                        Y = YS[g]
                        wr0, wr1 = LIVE[g]
                        chunks = [(a, min(a + 16, nr)) for a in range(0, nr, 16)]
                        for sx in range(2):
                            s = 2 * sy + sx
                            ps = ps_pool.tile([128, 1024], f32, name="ps")
                            for k in range(2):
                                lhs = i1s[k][s][:, 32 * Y : 32 * Y + 128]
                                for j, (a, b) in enumerate(chunks):
                                    r = Y + wr0 + a - 10  # 1st interior sub-row
                                    rhs = i2v[k][
                                        :,
                                        2 * r + sy : 2 * (r + b - a - 1) + sy + 1 : 2,
                                        sx::2,
                                    ]
                                    nc.tensor.matmul(
                                        ps[:, 512 * j : 512 * j + (b - a) * 32],
                                        lhsT=lhs,
                                        rhs=rhs,
                                        start=(k == 0),
                                        stop=(k == 1),
                                    )
                            # drain psum into the dense band slot
                            # (scale 1/C, cast to bf16)
                            q = 2 * gi + sx
                            dst = bt[:, q * slot : (q + 1) * slot]
                            src = ps[:, 0:slot]
                            if sx == 0:
                                nc.vector.tensor_scalar_mul(dst, src, inv_c)
                            else:
                                nc.scalar.mul(dst, src, inv_c)
                    # one linear dump per group: [128, 4*nr*32] bf16 -> HBM
                    n = GROUP_ELEMS[gp]
                    nc.sync.dma_start(out=od_t.ap()[off : off + n], in_=bt[:])
                    off += n

    nc.compile()
    return nc


def _get_program():
    if "nc" not in _CACHE:
        _CACHE["nc"] = _build()
    return _CACHE["nc"]


# host-side unpack indices (precomputed once)
_P = np.arange(128)
_PY = _P >> 5
_PX = _P & 31
_OY = np.arange(ND)
_OX = np.arange(ND)


def _unpack(dump: np.ndarray) -> np.ndarray:
    """[DUMP_ELEMS] bf16 -> [D, H, W] fp32 (pure permutation + zero pad)."""
    out = np.zeros((D, H, W), dtype=np.float32)
    ridx = _PY[:, None, None] + _OY[None, :, None]  # window row per (p, oy)
    cidx = _PX[:, None, None] + _OX[None, None, :]  # window col per (p, ox)
    off = 0
    for sy in range(2):
        for gp, pair in enumerate(GPAIR):
            nr = NRS[gp]
            n = GROUP_ELEMS[gp]
            blk = dump[off : off + n].reshape(128, 4, nr, 32)
            off += n
            for gi, g in enumerate(pair):
                Y = YS[g]
                wr0, _ = LIVE[g]
                for sx in range(2):
                    band = np.zeros((128, 24, 52), dtype=np.float32)
                    band[:, wr0 : wr0 + nr, 10:42] = blk[:, 2 * gi + sx]
                    vals = band[_P[:, None, None], ridx, cidx]  # [128, 21, 21]
                    ys = 2 * (Y + _PY) + sy
                    xs = 2 * _PX + sx
                    out[:, ys, xs] = vals.reshape(128, D).T
    return out


def kernel(input1: np.ndarray, input2: np.ndarray) -> np.ndarray:
    from concourse import bass_utils

    nc = _get_program()
    input1 = np.ascontiguousarray(input1, dtype=np.float32)
    input2 = np.ascontiguousarray(input2, dtype=np.float32)
    B = input1.shape[0]
    in_maps = [{"i1": input1[b], "i2": input2[b]} for b in range(B)]
    res = bass_utils.run_bass_kernel_spmd(nc, in_maps, core_ids=list(range(B)))
    out = np.stack(
        [
            _unpack(np.asarray(r["od"]).astype(np.float32))
            for r in res.results
        ]
    )
    return np.ascontiguousarray(out)  # [B, D, H, W]
